# revision 8
# baseline (speedup 1.0000x reference)
"""Multi-head self-attention Trainium2 kernel (8 NeuronCores).

Problem: B=4, N=2048, D=1024, H=16 heads of dim 64, fp32 in/out.

Sharding: 8 cores = 4 batches x 2 head-groups. Core c handles batch c//2
and heads (c%2)*8 .. (c%2)*8+7 (a 512-wide slice of the hidden dim).
Each core computes q/k/v projections for its head slice, attention for
its 8 heads, and a partial out-projection (contraction over its 512
attention dims). Host sums the two partials per batch.

Device dataflow (per core), all matmuls bf16 with fp32 PSUM accumulate:
  - x^T (host-pretransposed, bf16) lives in SBUF as 8 [128, 2048] tiles.
  - q_a/k_a = W^T.T @ x^T in "layout a" [head_dim-part, token-free].
  - v in "layout b" [token-part, head_dim-free], restrided into per-head
    65-column segments whose last column is ones (gives the softmax
    denominator for free during the PV matmul).
  - scores computed transposed: S^T[j, i] = k_a^T q_a (K=64 contraction),
    exp on ScalarE (scale=1/8 folded in, no max subtraction -- scores are
    ~N(0,1) so exp is safe), output P^T bf16 straight to SBUF.
  - PV: out[65, i] += v'[j,:65]^T P^T[j, i]; row 64 = sum_j exp = denom.
  - normalize: fast-approx reciprocal of row 64 (fp32 den staged at
    eviction), partition-broadcast, multiply.
  - out-projection: o[token, d_out] = attn^T.T @ Wo_slice^T, fp32 out.

Schedule: the exp stream on ScalarE is the critical resource (~285us of
ACT work vs ~300us PE work); every phase that would idle ACT is
interleaved into the attention j-loops through the shared PSUM ring:
next pair's kq projection chunks run as fillers inside the previous
pair's attention, and the first half of the out-projection runs as
fillers inside the last pair's second attention block.

Biases: bq applied on device (per-partition in layout a). bk cancels
exactly in softmax (adds a per-query constant to scores). bv and bo are
folded on host: attn rows sum to 1 so bv passes through linearly.
"""

import numpy as np
import ml_dtypes

BF16 = ml_dtypes.bfloat16

HIDDEN = 1024
N_TOK = 2048
BATCH = 4
N_CORES = 8

_CACHE = {}


def _build_nc(D, N):
    """Build + compile the per-core Bass program.

    Per-core tensor shapes (DL = D // 2 local q/k/v width):
      xT  [D, N]  bf16   : x[b] transposed
      wqT/wkT/wvT [D, DL] bf16 : W[hs:hs+DL, :].T
      woT [DL, D] bf16   : Wo[:, hs:hs+DL].T
      bqt [128, DL//128] f32 : bq slice, chunked per partition
      o   [N, D]  f32    : partial output (host sums pairs)
    """
    import concourse.bacc as bacc
    import concourse.mybir as mybir
    import concourse.tile as tile
    from contextlib import ExitStack

    dt = mybir.dt
    P = 128
    DL = D // 2
    KC = D // P          # d_model chunks (8)
    MC = DL // P         # head-dim chunks == head pairs (4)
    NHL = DL // 64       # local heads (8)
    NT = N // P          # token tiles (16)
    ICB = N // 2         # i-block width (1024)
    MMW = min(512, ICB)  # matmul moving width
    NSL = ICB // MMW     # moving slices per i-block (2)
    JT = NT              # j tiles (16)

    nc = bacc.Bacc("TRN2", target_bir_lowering=False, debug=False)

    xT = nc.dram_tensor("xT", [D, N], dt.bfloat16, kind="ExternalInput")
    wqT = nc.dram_tensor("wqT", [D, DL], dt.bfloat16, kind="ExternalInput")
    wkT = nc.dram_tensor("wkT", [D, DL], dt.bfloat16, kind="ExternalInput")
    wvT = nc.dram_tensor("wvT", [D, DL], dt.bfloat16, kind="ExternalInput")
    woT = nc.dram_tensor("woT", [DL, D], dt.bfloat16, kind="ExternalInput")
    bqt = nc.dram_tensor("bqt", [P, MC], dt.float32, kind="ExternalInput")
    o = nc.dram_tensor("o", [N, D], dt.float32, kind="ExternalOutput")

    with tile.TileContext(nc) as tc, ExitStack() as ctx:
        pers = ctx.enter_context(tc.tile_pool(name="pers", bufs=1))
        work = ctx.enter_context(tc.tile_pool(name="work", bufs=2))
        pmm = ctx.enter_context(tc.tile_pool(name="pmm", bufs=2, space="PSUM"))
        ppv = ctx.enter_context(tc.tile_pool(name="ppv", bufs=2, space="PSUM"))

        # ---- persistent SBUF tiles + input DMAs ----
        xt_t = [pers.tile([P, N], dt.bfloat16, name=f"xT{k}", tag=f"xT{k}") for k in range(KC)]
        wq_t = [pers.tile([P, DL], dt.bfloat16, name=f"wq{k}", tag=f"wq{k}") for k in range(KC)]
        wk_t = [pers.tile([P, DL], dt.bfloat16, name=f"wk{k}", tag=f"wk{k}") for k in range(KC)]
        wv_t = [pers.tile([P, DL], dt.bfloat16, name=f"wv{k}", tag=f"wv{k}") for k in range(KC)]
        wo_t = [pers.tile([P, D], dt.bfloat16, name=f"wo{m}", tag=f"wo{m}") for m in range(MC)]
        bq_t = pers.tile([P, MC], dt.float32, name="bqt_sb", tag="bqt")
        qa = [pers.tile([P, N], dt.bfloat16, name=f"qa{m}", tag=f"qa{m}") for m in range(MC)]
        ka = [pers.tile([P, N], dt.bfloat16, name=f"ka{m}", tag=f"ka{m}") for m in range(MC)]
        vp = [pers.tile([P, NHL * 65], dt.bfloat16, name=f"vp{t}", tag=f"vp{t}") for t in range(NT)]
        attn = [pers.tile([P, N], dt.bfloat16, name=f"attn{m}", tag=f"attn{m}") for m in range(MC)]

        # v_proj needs xT+wv first; then k/q weights (first attention pair);
        # wo/bq last (only needed at normalize/out-proj time).
        for k in range(KC):
            nc.sync.dma_start(xt_t[k][:], xT[k * P:(k + 1) * P, :])
            nc.sync.dma_start(wv_t[k][:], wvT[k * P:(k + 1) * P, :])
        for k in range(KC):
            nc.sync.dma_start(wk_t[k][:], wkT[k * P:(k + 1) * P, :])
            nc.sync.dma_start(wq_t[k][:], wqT[k * P:(k + 1) * P, :])
        for m in range(MC):
            nc.sync.dma_start(wo_t[m][:], woT[m * P:(m + 1) * P, :])
        nc.sync.dma_start(bq_t[:], bqt[:, :])

        def v_proj_tile(t):
            # v projection for token tile t: out [token, DL], restrided
            # into 65-col segments with a ones column per head.
            ps = pmm.tile([P, DL], dt.float32, tag="mm", name="psv")
            for k in range(KC):
                for s in range(0, DL, 512):
                    w = min(512, DL - s)
                    nc.tensor.matmul(
                        out=ps[:, s:s + w],
                        lhsT=xt_t[k][:, t * P:(t + 1) * P],
                        rhs=wv_t[k][:, s:s + w],
                        start=(k == 0),
                        stop=(k == KC - 1),
                    )
            seg = vp[t][:].rearrange("p (s c) -> p s c", c=65)
            nc.vector.memset(seg[:, :, 64:65], 1.0)
            nc.vector.tensor_copy(
                seg[:, :, 0:64],
                ps[:].rearrange("p (s c) -> p s c", c=64),
            )

        def kq_chunk(m, which, n2):
            """One quarter of a pair's kq projection: tensor `which`
            ('k'/'q') over i-block starting at n2. 16 matmuls into one
            PSUM ring slot, then a single eviction (cast, +bias for q)."""
            wt, bias, dst = (
                (wk_t, None, ka) if which == "k" else (wq_t, bq_t, qa))
            ps = pmm.tile([P, ICB], dt.float32, tag="mm", name="psp")
            for k in range(KC):
                for s in range(0, ICB, MMW):
                    nc.tensor.matmul(
                        out=ps[:, s:s + MMW],
                        lhsT=wt[k][:, m * P:(m + 1) * P],
                        rhs=xt_t[k][:, n2 + s:n2 + s + MMW],
                        start=(k == 0),
                        stop=(k == KC - 1),
                    )
            if bias is None:
                nc.vector.tensor_copy(dst[m][:, n2:n2 + ICB], ps[:])
            else:
                nc.vector.tensor_scalar_add(
                    dst[m][:, n2:n2 + ICB], ps[:], bias[:, m:m + 1])

        def attn_pair(m, ib, fillers=()):
            """Attention for head pair (2m, 2m+1) over i-block ib.

            Score matmuls for the two heads are emitted s-outer so the
            h0/h1 pair (K=64 stationaries at base partitions 0/64 ->
            disjoint PE row groups) sits adjacent in the PE queue and
            runs concurrently. `fillers` is a dict {j: callable} of
            extra work (kq chunks / out-proj tiles of other phases)
            emitted mid-loop so the PE uses the slack under the
            ACT-bound exp stream without ever starving it for long.
            Returns the unnormalized PV results copied to SBUF plus the
            fp32 denominator rows staged for the deferred normalize.
            """
            i0 = ib * ICB
            heads = (2 * m, 2 * m + 1)
            pvs = {}
            for h in heads:
                pvs[h] = ppv.tile([65, ICB], dt.float32, tag="pv", name="pv")
            for j in range(JT):
                if j in fillers:
                    fillers[j]()
                pss = {}
                for h in heads:
                    pss[h] = pmm.tile([P, ICB], dt.float32, tag="mm",
                                      name="pss")
                for s in range(0, ICB, MMW):
                    for h in heads:
                        r = (h % 2) * 64
                        nc.tensor.matmul(
                            out=pss[h][:, s:s + MMW],
                            lhsT=ka[m][r:r + 64, j * P:(j + 1) * P],
                            rhs=qa[m][r:r + 64, i0 + s:i0 + s + MMW],
                            start=True,
                            stop=True,
                        )
                pts = {}
                for h in heads:
                    pt = work.tile([P, ICB], dt.bfloat16, tag="pt",
                                   name="pt", bufs=8)
                    pts[h] = pt
                    nc.scalar.activation(
                        pt[:], pss[h][:],
                        mybir.ActivationFunctionType.Exp,
                        bias=0.0, scale=0.125,
                    )
                for h in heads:
                    for s in range(0, ICB, MMW):
                        nc.tensor.matmul(
                            out=pvs[h][:, s:s + MMW],
                            lhsT=vp[j][:, h * 65:(h + 1) * 65],
                            rhs=pts[h][:, s:s + MMW],
                            start=(j == 0),
                            stop=(j == JT - 1),
                        )
            out = {}
            for h in heads:
                pv_sb = work.tile([64, ICB], dt.bfloat16, tag="pvsb",
                                  name="pvsb", bufs=4)
                # fp32 denominator row staged for the fast reciprocal
                den = work.tile([1, ICB], dt.float32, tag="den",
                                name="den", bufs=4)
                nc.vector.tensor_copy(pv_sb[:], pvs[h][0:64, :])
                nc.vector.tensor_copy(den[:], pvs[h][64:65, :])
                out[h] = (pv_sb, den)
            return (m, ib, out)

        def normalize(pending):
            """Deferred softmax normalization, off the critical path.
            Fast approximate reciprocal (51 ULP) keeps the DVE queue
            short so PSUM evictions of interleaved phases never stall
            behind it."""
            m, ib, pv_sbs = pending
            i0 = ib * ICB
            for h, (pv_sb, den) in pv_sbs.items():
                r = (h % 2) * 64
                recip = work.tile([1, ICB], dt.float32, tag="recip",
                                  name="recip", bufs=2)
                nc.vector.reciprocal_approx_fast(recip[:], den[:])
                bcast = work.tile([64, ICB], dt.float32, tag="bcast",
                                  name="bcast")
                nc.gpsimd.partition_broadcast(bcast[:], recip[:])
                nc.vector.tensor_tensor(
                    attn[m][r:r + 64, i0:i0 + ICB],
                    pv_sb[:],
                    bcast[:],
                    mybir.AluOpType.mult,
                )

        def outproj(t, korder):
            ps = pmm.tile([P, D], dt.float32, tag="mm", name="pso")
            for ki, k in enumerate(korder):
                for s in range(0, D, 512):
                    w = min(512, D - s)
                    nc.tensor.matmul(
                        out=ps[:, s:s + w],
                        lhsT=attn[k][:, t * P:(t + 1) * P],
                        rhs=wo_t[k][:, s:s + w],
                        start=(ki == 0),
                        stop=(ki == MC - 1),
                    )
            oe = work.tile([P, D], dt.float32, tag="oev", name="oe")
            nc.vector.tensor_copy(oe[:], ps[:])
            nc.sync.dma_start(o[t * P:(t + 1) * P, :], oe[:])

        # Schedule. Pair order rotated so pair 0 is last; out-proj
        # contraction order matches so the last-normalized pair is
        # accumulated last. Startup: v projection (overlapping the
        # xT/wv DMA stream) then the first pair's kq. After that the
        # exp stream should never stop: each later pair's kq runs as
        # four filler chunks inside the previous pair's attention, and
        # the ib0 out-projections run as fillers inside the last
        # attention block.
        order = list(range(1, MC)) + [0]
        for t in range(NT):
            v_proj_tile(t)
        for which in ("k", "q"):
            for n2 in (0, ICB):
                kq_chunk(order[0], which, n2)

        prev = None
        for mi, m in enumerate(order):
            nxt = order[mi + 1] if mi + 1 < len(order) else None
            if nxt is not None:
                f0 = {5: lambda: kq_chunk(nxt, "k", 0),
                      11: lambda: kq_chunk(nxt, "k", ICB)}
                f1 = {5: lambda: kq_chunk(nxt, "q", 0),
                      11: lambda: kq_chunk(nxt, "q", ICB)}
            else:
                # Last pair: ib0 out-projections as fillers in its
                # second attention block (all ib0 normalizes land by
                # then; korder puts the freshest pair last).
                f0 = {}
                f1 = {j: (lambda t=t: outproj(t, order))
                      for j, t in zip((6, 8, 10, 12, 13, 14, 15),
                                      range(NT // 2 - 1))}
            a0 = attn_pair(m, 0, fillers=f0)
            if prev is not None:
                normalize(prev)
            if nxt is None:
                # Emit the last pair's ib0 normalize before its ib1
                # attention so the filler out-projections can run.
                normalize(a0)
                a1 = attn_pair(m, 1, fillers=f1)
            else:
                a1 = attn_pair(m, 1, fillers=f1)
                normalize(a0)
            prev = a1
        outproj(NT // 2 - 1, order)
        normalize(prev)
        for t in range(NT // 2, NT):
            outproj(t, order)

    nc.compile()
    return nc


def _get_nc(D, N):
    key = (D, N)
    if key not in _CACHE:
        _CACHE[key] = _build_nc(D, N)
    return _CACHE[key]


def _make_in_maps(x, Wq, bq, Wk, Wv, Wo, D, N):
    DL = D // 2
    MC = DL // 128
    in_maps = []
    for c in range(N_CORES):
        b = c // 2
        hs = (c % 2) * DL
        in_maps.append({
            "xT": np.ascontiguousarray(x[b].T).astype(BF16),
            "wqT": np.ascontiguousarray(Wq[hs:hs + DL, :].T).astype(BF16),
            "wkT": np.ascontiguousarray(Wk[hs:hs + DL, :].T).astype(BF16),
            "wvT": np.ascontiguousarray(Wv[hs:hs + DL, :].T).astype(BF16),
            "woT": np.ascontiguousarray(Wo[:, hs:hs + DL].T).astype(BF16),
            "bqt": np.ascontiguousarray(
                bq[hs:hs + DL].reshape(MC, 128).T).astype(np.float32),
        })
    return in_maps


def _run(x, Wq, bq, Wk, bk, Wv, bv, Wo, bo, trace=False):
    from concourse.bass_utils import run_bass_kernel_spmd

    x = np.asarray(x, np.float32)
    B, N, D = x.shape
    nc = _get_nc(D, N)
    in_maps = _make_in_maps(
        x, np.asarray(Wq, np.float32), np.asarray(bq, np.float32),
        np.asarray(Wk, np.float32), np.asarray(Wv, np.float32),
        np.asarray(Wo, np.float32), D, N)
    res = run_bass_kernel_spmd(
        nc, in_maps, list(range(N_CORES)), trace=trace)

    bv = np.asarray(bv, np.float32)
    bo = np.asarray(bo, np.float32)
    extra = bv @ np.asarray(Wo, np.float32).T + bo  # exact linear fold
    out = np.empty((B, N, D), np.float32)
    for b in range(B):
        out[b] = res.results[2 * b]["o"] + res.results[2 * b + 1]["o"] + extra
    return out, res


def kernel(x, Wq, bq, Wk, bk, Wv, bv, Wo, bo):
    out, _ = _run(x, Wq, bq, Wk, bk, Wv, bv, Wo, bo, trace=False)
    return out


# revision 10
# speedup vs baseline: 1.0088x; 1.0088x over previous
"""Multi-head self-attention Trainium2 kernel (8 NeuronCores).

Problem: B=4, N=2048, D=1024, H=16 heads of dim 64, fp32 in/out.

Sharding: 8 cores = 4 batches x 2 head-groups. Core c handles batch c//2
and heads (c%2)*8 .. (c%2)*8+7 (a 512-wide slice of the hidden dim).
Each core computes q/k/v projections for its head slice, attention for
its 8 heads, and a partial out-projection (contraction over its 512
attention dims). Host sums the two partials per batch.

Device dataflow (per core), all matmuls bf16 with fp32 PSUM accumulate:
  - x^T (host-pretransposed, bf16) lives in SBUF as 8 [128, 2048] tiles.
  - q_a/k_a = W^T.T @ x^T in "layout a" [head_dim-part, token-free].
  - v in "layout b" [token-part, head_dim-free], restrided into per-head
    65-column segments whose last column is ones (gives the softmax
    denominator for free during the PV matmul).
  - scores computed transposed: S^T[j, i] = k_a^T q_a (K=64 contraction),
    exp on ScalarE (scale=1/8 folded in, no max subtraction -- scores are
    ~N(0,1) so exp is safe), output P^T bf16 straight to SBUF.
  - PV: out[65, i] += v'[j,:65]^T P^T[j, i]; row 64 = sum_j exp = denom.
  - normalize: fast-approx reciprocal of the denominator row,
    partition-broadcast, multiply.
  - out-projection: o[token, d_out] = attn^T.T @ Wo_slice^T, fp32 out.

Schedule: the exp stream on ScalarE (~285us) is the critical resource;
the PE work (~330us serial) must hide under it. Startup runs the v
projection k-outer in two 8-tile waves that overlap the input DMA
stream and keep the PE dense (HAM stays warm), then the first pair's
kq. Every later pair's kq runs as four filler chunks inside the
previous pair's attention j-loop; the two j-tiles of scores before
each filler are cast to SBUF (DVE) so the exp stream has a runway to
chew through while the filler holds the PE and the PSUM ring. The
first-half out-projections run as fillers (with runway) inside the
last attention block; the last unit normalizes directly from PSUM so
the tail out-projections start immediately.

Biases: bq applied on device (per-partition in layout a). bk cancels
exactly in softmax (adds a per-query constant to scores). bv and bo are
folded on host: attn rows sum to 1 so bv passes through linearly.
"""

import numpy as np
import ml_dtypes

BF16 = ml_dtypes.bfloat16

HIDDEN = 1024
N_TOK = 2048
BATCH = 4
N_CORES = 8

_CACHE = {}


def _build_nc(D, N):
    """Build + compile the per-core Bass program.

    Per-core tensor shapes (DL = D // 2 local q/k/v width):
      xT  [D, N]  bf16   : x[b] transposed
      wqT/wkT/wvT [D, DL] bf16 : W[hs:hs+DL, :].T
      woT [DL, D] bf16   : Wo[:, hs:hs+DL].T
      bqt [128, DL//128] f32 : bq slice, chunked per partition
      o   [N, D]  f32    : partial output (host sums pairs)
    """
    import concourse.bacc as bacc
    import concourse.mybir as mybir
    import concourse.tile as tile
    from contextlib import ExitStack

    dt = mybir.dt
    P = 128
    DL = D // 2
    KC = D // P          # d_model chunks (8)
    MC = DL // P         # head-dim chunks == head pairs (4)
    NHL = DL // 64       # local heads (8)
    NT = N // P          # token tiles (16)
    ICB = N // 2         # i-block width (1024)
    MMW = min(512, ICB)  # matmul moving width
    JT = NT              # j tiles (16)

    nc = bacc.Bacc("TRN2", target_bir_lowering=False, debug=False)

    xT = nc.dram_tensor("xT", [D, N], dt.bfloat16, kind="ExternalInput")
    wqT = nc.dram_tensor("wqT", [D, DL], dt.bfloat16, kind="ExternalInput")
    wkT = nc.dram_tensor("wkT", [D, DL], dt.bfloat16, kind="ExternalInput")
    wvT = nc.dram_tensor("wvT", [D, DL], dt.bfloat16, kind="ExternalInput")
    woT = nc.dram_tensor("woT", [DL, D], dt.bfloat16, kind="ExternalInput")
    bqt = nc.dram_tensor("bqt", [P, MC], dt.float32, kind="ExternalInput")
    o = nc.dram_tensor("o", [N, D], dt.float32, kind="ExternalOutput")

    with tile.TileContext(nc) as tc, ExitStack() as ctx:
        pers = ctx.enter_context(tc.tile_pool(name="pers", bufs=1))
        work = ctx.enter_context(tc.tile_pool(name="work", bufs=2))
        pmm = ctx.enter_context(tc.tile_pool(name="pmm", bufs=2, space="PSUM"))
        ppv = ctx.enter_context(tc.tile_pool(name="ppv", bufs=2, space="PSUM"))

        # ---- persistent SBUF tiles + input DMAs ----
        xt_t = [pers.tile([P, N], dt.bfloat16, name=f"xT{k}", tag=f"xT{k}") for k in range(KC)]
        wq_t = [pers.tile([P, DL], dt.bfloat16, name=f"wq{k}", tag=f"wq{k}") for k in range(KC)]
        wk_t = [pers.tile([P, DL], dt.bfloat16, name=f"wk{k}", tag=f"wk{k}") for k in range(KC)]
        wv_t = [pers.tile([P, DL], dt.bfloat16, name=f"wv{k}", tag=f"wv{k}") for k in range(KC)]
        wo_t = [pers.tile([P, D], dt.bfloat16, name=f"wo{m}", tag=f"wo{m}") for m in range(MC)]
        bq_t = pers.tile([P, MC], dt.float32, name="bqt_sb", tag="bqt")
        qa = [pers.tile([P, N], dt.bfloat16, name=f"qa{m}", tag=f"qa{m}") for m in range(MC)]
        ka = [pers.tile([P, N], dt.bfloat16, name=f"ka{m}", tag=f"ka{m}") for m in range(MC)]
        vp = [pers.tile([P, NHL * 65], dt.bfloat16, name=f"vp{t}", tag=f"vp{t}") for t in range(NT)]
        attn = [pers.tile([P, N], dt.bfloat16, name=f"attn{m}", tag=f"attn{m}") for m in range(MC)]

        # v_proj needs xT+wv first; then k/q weights (first attention
        # pair); wo/bq last (needed only at out-proj time).
        for k in range(KC):
            nc.sync.dma_start(xt_t[k][:], xT[k * P:(k + 1) * P, :])
            nc.sync.dma_start(wv_t[k][:], wvT[k * P:(k + 1) * P, :])
        for k in range(KC):
            nc.sync.dma_start(wk_t[k][:], wkT[k * P:(k + 1) * P, :])
            nc.sync.dma_start(wq_t[k][:], wqT[k * P:(k + 1) * P, :])
        for m in range(MC):
            nc.sync.dma_start(wo_t[m][:], woT[m * P:(m + 1) * P, :])
        nc.sync.dma_start(bq_t[:], bqt[:, :])

        def v_proj_wave(ts):
            """v projection for 8 token tiles, k-outer: accumulation
            runs in lockstep with the xT/wv DMA arrival order so the PE
            starts immediately and stays dense (HAM warms up early).
            Two token tiles pack into each [128, 1024] PSUM slot; the
            wave borrows both pools' slots (attention hasn't started)."""
            tiles = []
            for g in range(0, len(ts), 2):
                pool, tg = (pmm, "mm") if g < 4 else (ppv, "pv")
                tiles.append(pool.tile([P, 2 * DL], dt.float32, tag=tg,
                                       name="psv"))
            for k in range(KC):
                for g, tl in enumerate(tiles):
                    for u in (0, 1):
                        t = ts[2 * g + u]
                        nc.tensor.matmul(
                            out=tl[:, u * DL:(u + 1) * DL],
                            lhsT=xt_t[k][:, t * P:(t + 1) * P],
                            rhs=wv_t[k][:],
                            start=(k == 0),
                            stop=(k == KC - 1),
                        )
            for g, tl in enumerate(tiles):
                for u in (0, 1):
                    t = ts[2 * g + u]
                    seg = vp[t][:].rearrange("p (s c) -> p s c", c=65)
                    nc.vector.memset(seg[:, :, 64:65], 1.0)
                    nc.vector.tensor_copy(
                        seg[:, :, 0:64],
                        tl[:, u * DL:(u + 1) * DL].rearrange(
                            "p (s c) -> p s c", c=64),
                    )

        def kq_chunk(m, which, n2):
            """One quarter of a pair's kq projection: tensor `which`
            ('k'/'q') over i-block starting at n2. 16 matmuls into one
            PSUM ring slot, then a single eviction (cast, +bias for q)."""
            wt, bias, dst = (
                (wk_t, None, ka) if which == "k" else (wq_t, bq_t, qa))
            ps = pmm.tile([P, ICB], dt.float32, tag="mm", name="psp")
            for k in range(KC):
                for s in range(0, ICB, MMW):
                    nc.tensor.matmul(
                        out=ps[:, s:s + MMW],
                        lhsT=wt[k][:, m * P:(m + 1) * P],
                        rhs=xt_t[k][:, n2 + s:n2 + s + MMW],
                        start=(k == 0),
                        stop=(k == KC - 1),
                    )
            if bias is None:
                nc.vector.tensor_copy(dst[m][:, n2:n2 + ICB], ps[:])
            else:
                nc.vector.tensor_scalar_add(
                    dst[m][:, n2:n2 + ICB], ps[:], bias[:, m:m + 1])

        def attn_pair(m, ib, fillers=(), cast_js=(), evict=True):
            """Attention for head pair (2m, 2m+1) over i-block ib.

            fillers: {j: callable} -- foreign PE work (kq chunks /
            out-projections) emitted mid-loop. cast_js: j values whose
            scores are staged to SBUF by the DVE before the exp; this
            builds an exp-input runway so ScalarE keeps running while a
            filler occupies the PE and the PSUM ring (and so the PE is
            never idle long enough to trip the HAM re-throttle).
            evict=False returns live PSUM handles for the final unit
            (normalized straight from PSUM, no staging copies).
            """
            i0 = ib * ICB
            heads = (2 * m, 2 * m + 1)
            pvs = {}
            for h in heads:
                pvs[h] = ppv.tile([P, ICB], dt.float32, tag="pv", name="pv")
            for j in range(JT):
                if j in fillers:
                    fillers[j]()
                pss = {}
                for h in heads:
                    pss[h] = pmm.tile([P, ICB], dt.float32, tag="mm",
                                      name="pss")
                for h in heads:
                    r = (h % 2) * 64
                    for s in range(0, ICB, MMW):
                        nc.tensor.matmul(
                            out=pss[h][:, s:s + MMW],
                            lhsT=ka[m][r:r + 64, j * P:(j + 1) * P],
                            rhs=qa[m][r:r + 64, i0 + s:i0 + s + MMW],
                            start=True,
                            stop=True,
                        )
                pts = {}
                for h in heads:
                    src = pss[h]
                    if j in cast_js:
                        sb = work.tile([P, ICB], dt.bfloat16, tag="scrw",
                                       name="scrw", bufs=4)
                        nc.vector.tensor_copy(sb[:], pss[h][:])
                        src = sb
                    pt = work.tile([P, ICB], dt.bfloat16, tag="pt",
                                   name="pt", bufs=8)
                    pts[h] = pt
                    nc.scalar.activation(
                        pt[:], src[:],
                        mybir.ActivationFunctionType.Exp,
                        bias=0.0, scale=0.125,
                    )
                for h in heads:
                    for s in range(0, ICB, MMW):
                        nc.tensor.matmul(
                            out=pvs[h][0:65, s:s + MMW],
                            lhsT=vp[j][:, h * 65:(h + 1) * 65],
                            rhs=pts[h][:, s:s + MMW],
                            start=(j == 0),
                            stop=(j == JT - 1),
                        )
            if not evict:
                return (m, ib, pvs)
            out = {}
            for h in heads:
                pv_sb = work.tile([64, ICB], dt.bfloat16, tag="pvsb",
                                  name="pvsb", bufs=4)
                den = work.tile([1, ICB], dt.float32, tag="den",
                                name="den", bufs=4)
                nc.vector.tensor_copy(pv_sb[:], pvs[h][0:64, :])
                nc.vector.tensor_copy(den[:], pvs[h][64:65, :])
                out[h] = (pv_sb, den)
            return (m, ib, out)

        def normalize(pending, direct=False):
            """Deferred softmax normalization, off the critical path.
            direct=True reads the live PSUM tiles (final unit only)."""
            m, ib, srcs = pending
            i0 = ib * ICB
            for h, src in srcs.items():
                r = (h % 2) * 64
                if direct:
                    num, den = src[0:64, :], src[64:65, :]
                else:
                    num, den = src[0], src[1][:]
                recip = work.tile([1, ICB], dt.float32, tag="recip",
                                  name="recip", bufs=2)
                nc.vector.reciprocal_approx_fast(recip[:], den)
                bcast = work.tile([64, ICB], dt.float32, tag="bcast",
                                  name="bcast")
                nc.gpsimd.partition_broadcast(bcast[:], recip[:])
                nc.vector.tensor_tensor(
                    attn[m][r:r + 64, i0:i0 + ICB],
                    num,
                    bcast[:],
                    mybir.AluOpType.mult,
                )

        def outproj(t, korder):
            ps = pmm.tile([P, D], dt.float32, tag="mm", name="pso")
            for ki, k in enumerate(korder):
                for s in range(0, D, 512):
                    w = min(512, D - s)
                    nc.tensor.matmul(
                        out=ps[:, s:s + w],
                        lhsT=attn[k][:, t * P:(t + 1) * P],
                        rhs=wo_t[k][:, s:s + w],
                        start=(ki == 0),
                        stop=(ki == MC - 1),
                    )
            oe = work.tile([P, D], dt.float32, tag="oev", name="oe")
            nc.vector.tensor_copy(oe[:], ps[:])
            nc.sync.dma_start(o[t * P:(t + 1) * P, :], oe[:])

        # Schedule. Pair order rotated so pair 0 is last; out-proj
        # contraction order matches so the last-normalized pair is
        # accumulated last.
        order = list(range(1, MC)) + [0]
        v_proj_wave(list(range(0, 8)))
        v_proj_wave(list(range(8, NT)))
        for which in ("k", "q"):
            for n2 in (0, ICB):
                kq_chunk(order[0], which, n2)

        KQ_F = {5, 11}
        KQ_C = {3, 4, 9, 10}
        prev = None
        for mi, m in enumerate(order):
            nxt = order[mi + 1] if mi + 1 < len(order) else None
            if nxt is not None:
                f0 = {5: lambda: kq_chunk(nxt, "k", 0),
                      11: lambda: kq_chunk(nxt, "k", ICB)}
                f1 = {5: lambda: kq_chunk(nxt, "q", 0),
                      11: lambda: kq_chunk(nxt, "q", ICB)}
                c0 = c1 = KQ_C
                a0 = attn_pair(m, 0, fillers=f0, cast_js=c0)
                if prev is not None:
                    normalize(prev)
                a1 = attn_pair(m, 1, fillers=f1, cast_js=c1)
                normalize(a0)
                prev = a1
            else:
                # Last pair: ib0 out-projections as fillers (with a
                # runway cast before each) in its second attention
                # block; the final unit skips staging and normalizes
                # straight from PSUM so the tail starts immediately.
                f1 = {j: (lambda t=t: outproj(t, order))
                      for j, t in zip((2, 4, 6, 8, 10, 12, 14),
                                      range(NT // 2 - 1))}
                c1 = {1, 3, 5, 7, 9, 11, 13}
                a0 = attn_pair(m, 0)
                if prev is not None:
                    normalize(prev)
                normalize(a0)
                a1 = attn_pair(m, 1, fillers=f1, cast_js=c1)
                normalize(a1)
        for t in range(NT // 2 - 1, NT):
            outproj(t, order)

    nc.compile()
    return nc


def _get_nc(D, N):
    key = (D, N)
    if key not in _CACHE:
        _CACHE[key] = _build_nc(D, N)
    return _CACHE[key]


def _make_in_maps(x, Wq, bq, Wk, Wv, Wo, D, N):
    DL = D // 2
    MC = DL // 128
    in_maps = []
    for c in range(N_CORES):
        b = c // 2
        hs = (c % 2) * DL
        in_maps.append({
            "xT": np.ascontiguousarray(x[b].T).astype(BF16),
            "wqT": np.ascontiguousarray(Wq[hs:hs + DL, :].T).astype(BF16),
            "wkT": np.ascontiguousarray(Wk[hs:hs + DL, :].T).astype(BF16),
            "wvT": np.ascontiguousarray(Wv[hs:hs + DL, :].T).astype(BF16),
            "woT": np.ascontiguousarray(Wo[:, hs:hs + DL].T).astype(BF16),
            "bqt": np.ascontiguousarray(
                bq[hs:hs + DL].reshape(MC, 128).T).astype(np.float32),
        })
    return in_maps


def _run(x, Wq, bq, Wk, bk, Wv, bv, Wo, bo, trace=False):
    from concourse.bass_utils import run_bass_kernel_spmd

    x = np.asarray(x, np.float32)
    B, N, D = x.shape
    nc = _get_nc(D, N)
    in_maps = _make_in_maps(
        x, np.asarray(Wq, np.float32), np.asarray(bq, np.float32),
        np.asarray(Wk, np.float32), np.asarray(Wv, np.float32),
        np.asarray(Wo, np.float32), D, N)
    res = run_bass_kernel_spmd(
        nc, in_maps, list(range(N_CORES)), trace=trace)

    bv = np.asarray(bv, np.float32)
    bo = np.asarray(bo, np.float32)
    extra = bv @ np.asarray(Wo, np.float32).T + bo  # exact linear fold
    out = np.empty((B, N, D), np.float32)
    for b in range(B):
        out[b] = res.results[2 * b]["o"] + res.results[2 * b + 1]["o"] + extra
    return out, res


def kernel(x, Wq, bq, Wk, bk, Wv, bv, Wo, bo):
    out, _ = _run(x, Wq, bq, Wk, bk, Wv, bv, Wo, bo, trace=False)
    return out


# revision 12
# speedup vs baseline: 1.0092x; 1.0004x over previous
"""Multi-head self-attention Trainium2 kernel (8 NeuronCores).

Problem: B=4, N=2048, D=1024, H=16 heads of dim 64, fp32 in/out.

Sharding: 8 cores = 4 batches x 2 head-groups. Core c handles batch c//2
and heads (c%2)*8 .. (c%2)*8+7 (a 512-wide slice of the hidden dim).
Each core computes q/k/v projections for its head slice, attention for
its 8 heads, and a partial out-projection (contraction over its 512
attention dims). Host sums the two partials per batch.

Device dataflow (per core), all matmuls bf16 with fp32 PSUM accumulate:
  - x^T (host-pretransposed, bf16) lives in SBUF as 8 [128, 2048] tiles.
  - q_a/k_a = W^T.T @ x^T in "layout a" [head_dim-part, token-free].
  - v in "layout b" [token-part, head_dim-free], restrided into per-head
    65-column segments whose last column is ones (gives the softmax
    denominator for free during the PV matmul).
  - scores computed transposed: S^T[j, i] = k_a^T q_a (K=64 contraction),
    exp on ScalarE (scale=1/8 folded in, no max subtraction -- scores are
    ~N(0,1) so exp is safe), output P^T bf16 straight to SBUF.
  - PV: out[65, i] += v'[j,:65]^T P^T[j, i]; row 64 = sum_j exp = denom.
  - normalize: fast-approx reciprocal of the denominator row,
    partition-broadcast, multiply.
  - out-projection: o[token, d_out] = attn^T.T @ Wo_slice^T, fp32 out.

Schedule: the exp stream on ScalarE (~285us) is the critical resource;
the PE work (~330us serial) must hide under it. Startup runs the v
projection k-outer in two 8-tile waves that overlap the input DMA
stream and keep the PE dense (HAM stays warm), then the first pair's
kq. Every later pair's kq runs as four filler chunks inside the
previous pair's attention j-loop; the two j-tiles of scores before
each filler are cast to SBUF (DVE) so the exp stream has a runway to
chew through while the filler holds the PE and the PSUM ring. The
first-half out-projections run as fillers (with runway) inside the
last attention block; the last unit normalizes directly from PSUM so
the tail out-projections start immediately.

Biases: bq applied on device (per-partition in layout a). bk cancels
exactly in softmax (adds a per-query constant to scores). bv and bo are
folded on host: attn rows sum to 1 so bv passes through linearly.
"""

import numpy as np
import ml_dtypes

BF16 = ml_dtypes.bfloat16

HIDDEN = 1024
N_TOK = 2048
BATCH = 4
N_CORES = 8

_CACHE = {}


def _build_nc(D, N):
    """Build + compile the per-core Bass program.

    Per-core tensor shapes (DL = D // 2 local q/k/v width):
      xT  [D, N]  bf16   : x[b] transposed
      wqT/wkT/wvT [D, DL] bf16 : W[hs:hs+DL, :].T
      woT [DL, D] bf16   : Wo[:, hs:hs+DL].T
      bqt [128, DL//128] f32 : bq slice, chunked per partition
      o   [N, D]  f32    : partial output (host sums pairs)
    """
    import concourse.bacc as bacc
    import concourse.mybir as mybir
    import concourse.tile as tile
    from contextlib import ExitStack

    dt = mybir.dt
    P = 128
    DL = D // 2
    KC = D // P          # d_model chunks (8)
    MC = DL // P         # head-dim chunks == head pairs (4)
    NHL = DL // 64       # local heads (8)
    NT = N // P          # token tiles (16)
    ICB = N // 2         # i-block width (1024)
    MMW = min(512, ICB)  # matmul moving width
    JT = NT              # j tiles (16)

    nc = bacc.Bacc("TRN2", target_bir_lowering=False, debug=False)

    xT = nc.dram_tensor("xT", [D, N], dt.bfloat16, kind="ExternalInput")
    wqT = nc.dram_tensor("wqT", [D, DL], dt.bfloat16, kind="ExternalInput")
    wkT = nc.dram_tensor("wkT", [D, DL], dt.bfloat16, kind="ExternalInput")
    wvT = nc.dram_tensor("wvT", [D, DL], dt.bfloat16, kind="ExternalInput")
    woT = nc.dram_tensor("woT", [DL, D], dt.bfloat16, kind="ExternalInput")
    bqt = nc.dram_tensor("bqt", [P, MC], dt.float32, kind="ExternalInput")
    o = nc.dram_tensor("o", [N, D], dt.float32, kind="ExternalOutput")

    with tile.TileContext(nc) as tc, ExitStack() as ctx:
        pers = ctx.enter_context(tc.tile_pool(name="pers", bufs=1))
        work = ctx.enter_context(tc.tile_pool(name="work", bufs=2))
        pmm = ctx.enter_context(tc.tile_pool(name="pmm", bufs=2, space="PSUM"))
        ppv = ctx.enter_context(tc.tile_pool(name="ppv", bufs=2, space="PSUM"))

        # ---- persistent SBUF tiles + input DMAs ----
        xt_t = [pers.tile([P, N], dt.bfloat16, name=f"xT{k}", tag=f"xT{k}") for k in range(KC)]
        wq_t = [pers.tile([P, DL], dt.bfloat16, name=f"wq{k}", tag=f"wq{k}") for k in range(KC)]
        wk_t = [pers.tile([P, DL], dt.bfloat16, name=f"wk{k}", tag=f"wk{k}") for k in range(KC)]
        wv_t = [pers.tile([P, DL], dt.bfloat16, name=f"wv{k}", tag=f"wv{k}") for k in range(KC)]
        wo_t = [pers.tile([P, D], dt.bfloat16, name=f"wo{m}", tag=f"wo{m}") for m in range(MC)]
        bq_t = pers.tile([P, MC], dt.float32, name="bqt_sb", tag="bqt")
        qa = [pers.tile([P, N], dt.bfloat16, name=f"qa{m}", tag=f"qa{m}") for m in range(MC)]
        ka = [pers.tile([P, N], dt.bfloat16, name=f"ka{m}", tag=f"ka{m}") for m in range(MC)]
        vp = [pers.tile([P, NHL * 65], dt.bfloat16, name=f"vp{t}", tag=f"vp{t}") for t in range(NT)]
        attn = [pers.tile([P, N], dt.bfloat16, name=f"attn{m}", tag=f"attn{m}") for m in range(MC)]

        # v_proj needs xT+wv first; then k/q weights (first attention
        # pair); wo/bq last (needed only at out-proj time).
        for k in range(KC):
            nc.sync.dma_start(xt_t[k][:], xT[k * P:(k + 1) * P, :])
            nc.sync.dma_start(wv_t[k][:], wvT[k * P:(k + 1) * P, :])
        for k in range(KC):
            nc.sync.dma_start(wk_t[k][:], wkT[k * P:(k + 1) * P, :])
            nc.sync.dma_start(wq_t[k][:], wqT[k * P:(k + 1) * P, :])
        for m in range(MC):
            nc.sync.dma_start(wo_t[m][:], woT[m * P:(m + 1) * P, :])
        nc.sync.dma_start(bq_t[:], bqt[:, :])

        def v_proj_wave(ts):
            """v projection for 8 token tiles, k-outer: accumulation
            runs in lockstep with the xT/wv DMA arrival order so the PE
            starts immediately and stays dense (HAM warms up early).
            Two token tiles pack into each [128, 1024] PSUM slot; the
            wave borrows both pools' slots (attention hasn't started)."""
            tiles = []
            for g in range(0, len(ts), 2):
                pool, tg = (pmm, "mm") if g < 4 else (ppv, "pv")
                tiles.append(pool.tile([P, 2 * DL], dt.float32, tag=tg,
                                       name="psv"))
            for k in range(KC):
                for g, tl in enumerate(tiles):
                    for u in (0, 1):
                        t = ts[2 * g + u]
                        nc.tensor.matmul(
                            out=tl[:, u * DL:(u + 1) * DL],
                            lhsT=xt_t[k][:, t * P:(t + 1) * P],
                            rhs=wv_t[k][:],
                            start=(k == 0),
                            stop=(k == KC - 1),
                        )
            for g, tl in enumerate(tiles):
                for u in (0, 1):
                    t = ts[2 * g + u]
                    seg = vp[t][:].rearrange("p (s c) -> p s c", c=65)
                    nc.vector.memset(seg[:, :, 64:65], 1.0)
                    nc.vector.tensor_copy(
                        seg[:, :, 0:64],
                        tl[:, u * DL:(u + 1) * DL].rearrange(
                            "p (s c) -> p s c", c=64),
                    )

        def kq_chunk(m, which, n2):
            """One quarter of a pair's kq projection: tensor `which`
            ('k'/'q') over i-block starting at n2. 16 matmuls into one
            PSUM ring slot, then a single eviction (cast, +bias for q)."""
            wt, bias, dst = (
                (wk_t, None, ka) if which == "k" else (wq_t, bq_t, qa))
            ps = pmm.tile([P, ICB], dt.float32, tag="mm", name="psp")
            for k in range(KC):
                for s in range(0, ICB, MMW):
                    nc.tensor.matmul(
                        out=ps[:, s:s + MMW],
                        lhsT=wt[k][:, m * P:(m + 1) * P],
                        rhs=xt_t[k][:, n2 + s:n2 + s + MMW],
                        start=(k == 0),
                        stop=(k == KC - 1),
                    )
            if bias is None:
                nc.vector.tensor_copy(dst[m][:, n2:n2 + ICB], ps[:])
            else:
                nc.vector.tensor_scalar_add(
                    dst[m][:, n2:n2 + ICB], ps[:], bias[:, m:m + 1])

        def attn_pair(m, ib, fillers=()):
            """Attention for head pair (2m, 2m+1) over i-block ib.

            fillers: {j: callable} -- foreign PE work (out-projections
            of already-normalized token tiles) emitted mid-loop to use
            the PE slack under the ACT-bound exp stream.
            """
            i0 = ib * ICB
            heads = (2 * m, 2 * m + 1)
            pvs = {}
            for h in heads:
                pvs[h] = ppv.tile([P, ICB], dt.float32, tag="pv", name="pv")
            for j in range(JT):
                if j in fillers:
                    fillers[j]()
                pss = {}
                for h in heads:
                    pss[h] = pmm.tile([P, ICB], dt.float32, tag="mm",
                                      name="pss")
                for h in heads:
                    r = (h % 2) * 64
                    for s in range(0, ICB, MMW):
                        nc.tensor.matmul(
                            out=pss[h][:, s:s + MMW],
                            lhsT=ka[m][r:r + 64, j * P:(j + 1) * P],
                            rhs=qa[m][r:r + 64, i0 + s:i0 + s + MMW],
                            start=True,
                            stop=True,
                        )
                pts = {}
                for h in heads:
                    pt = work.tile([P, ICB], dt.bfloat16, tag="pt",
                                   name="pt", bufs=8)
                    pts[h] = pt
                    nc.scalar.activation(
                        pt[:], pss[h][:],
                        mybir.ActivationFunctionType.Exp,
                        bias=0.0, scale=0.125,
                    )
                for h in heads:
                    for s in range(0, ICB, MMW):
                        nc.tensor.matmul(
                            out=pvs[h][0:65, s:s + MMW],
                            lhsT=vp[j][:, h * 65:(h + 1) * 65],
                            rhs=pts[h][:, s:s + MMW],
                            start=(j == 0),
                            stop=(j == JT - 1),
                        )
            out = {}
            for h in heads:
                pv_sb = work.tile([64, ICB], dt.bfloat16, tag="pvsb",
                                  name="pvsb", bufs=4)
                den = work.tile([1, ICB], dt.float32, tag="den",
                                name="den", bufs=4)
                nc.vector.tensor_copy(pv_sb[:], pvs[h][0:64, :])
                nc.vector.tensor_copy(den[:], pvs[h][64:65, :])
                out[h] = (pv_sb, den)
            return (m, ib, out)

        def normalize(pending):
            """Deferred softmax normalization, off the critical path."""
            m, ib, srcs = pending
            i0 = ib * ICB
            for h, (pv_sb, den) in srcs.items():
                r = (h % 2) * 64
                recip = work.tile([1, ICB], dt.float32, tag="recip",
                                  name="recip", bufs=2)
                nc.vector.reciprocal_approx_fast(recip[:], den[:])
                bcast = work.tile([64, ICB], dt.float32, tag="bcast",
                                  name="bcast")
                nc.gpsimd.partition_broadcast(bcast[:], recip[:])
                nc.vector.tensor_tensor(
                    attn[m][r:r + 64, i0:i0 + ICB],
                    pv_sb[:],
                    bcast[:],
                    mybir.AluOpType.mult,
                )

        def outproj(t, korder):
            ps = pmm.tile([P, D], dt.float32, tag="mm", name="pso")
            for ki, k in enumerate(korder):
                for s in range(0, D, 512):
                    w = min(512, D - s)
                    nc.tensor.matmul(
                        out=ps[:, s:s + w],
                        lhsT=attn[k][:, t * P:(t + 1) * P],
                        rhs=wo_t[k][:, s:s + w],
                        start=(ki == 0),
                        stop=(ki == MC - 1),
                    )
            oe = work.tile([P, D], dt.float32, tag="oev", name="oe")
            nc.vector.tensor_copy(oe[:], ps[:])
            nc.sync.dma_start(o[t * P:(t + 1) * P, :], oe[:])

        # Schedule. Startup: v projection in two k-outer waves that
        # overlap the xT/wv DMA stream, then all four pairs' kq chunks
        # (dense warm PE, overlapping the weight DMA tail). Attention
        # blocks run all ib0 units first, then all ib1 units, so every
        # ib0 normalize lands before the ib1 phase and the ib0
        # out-projections spread across the four ib1 blocks as light
        # fillers (2 per block). Pair order rotated so pair 0 is last;
        # out-proj contraction order matches so the last-normalized
        # pair is accumulated last.
        order = list(range(1, MC)) + [0]
        v_proj_wave(list(range(0, 8)))
        v_proj_wave(list(range(8, NT)))
        for m in order:
            for which in ("k", "q"):
                for n2 in (0, ICB):
                    kq_chunk(m, which, n2)

        blocks = [(m, 0) for m in order] + [(m, 1) for m in order]
        pending = []
        ib0_t = list(range(NT // 2))  # token tiles covered by fillers
        for bi, (m, ib) in enumerate(blocks):
            fill = {}
            if ib == 1:
                f_ts = ib0_t[2 * (bi - MC):2 * (bi - MC) + 2]
                fill = {j: (lambda t=t: outproj(t, order))
                        for j, t in zip((5, 11), f_ts)}
            res = attn_pair(m, ib, fillers=fill)
            if pending:
                normalize(pending.pop(0))
            pending.append(res)
            if bi == MC - 1:
                # Last ib0 block: flush so all ib0 normalizes precede
                # the first ib1 block's out-projection fillers.
                while pending:
                    normalize(pending.pop(0))
        while pending:
            normalize(pending.pop(0))
        for t in range(NT // 2, NT):
            outproj(t, order)

    nc.compile()
    return nc


def _get_nc(D, N):
    key = (D, N)
    if key not in _CACHE:
        _CACHE[key] = _build_nc(D, N)
    return _CACHE[key]


def _make_in_maps(x, Wq, bq, Wk, Wv, Wo, D, N):
    DL = D // 2
    MC = DL // 128
    in_maps = []
    for c in range(N_CORES):
        b = c // 2
        hs = (c % 2) * DL
        in_maps.append({
            "xT": np.ascontiguousarray(x[b].T).astype(BF16),
            "wqT": np.ascontiguousarray(Wq[hs:hs + DL, :].T).astype(BF16),
            "wkT": np.ascontiguousarray(Wk[hs:hs + DL, :].T).astype(BF16),
            "wvT": np.ascontiguousarray(Wv[hs:hs + DL, :].T).astype(BF16),
            "woT": np.ascontiguousarray(Wo[:, hs:hs + DL].T).astype(BF16),
            "bqt": np.ascontiguousarray(
                bq[hs:hs + DL].reshape(MC, 128).T).astype(np.float32),
        })
    return in_maps


def _run(x, Wq, bq, Wk, bk, Wv, bv, Wo, bo, trace=False):
    from concourse.bass_utils import run_bass_kernel_spmd

    x = np.asarray(x, np.float32)
    B, N, D = x.shape
    nc = _get_nc(D, N)
    in_maps = _make_in_maps(
        x, np.asarray(Wq, np.float32), np.asarray(bq, np.float32),
        np.asarray(Wk, np.float32), np.asarray(Wv, np.float32),
        np.asarray(Wo, np.float32), D, N)
    res = run_bass_kernel_spmd(
        nc, in_maps, list(range(N_CORES)), trace=trace)

    bv = np.asarray(bv, np.float32)
    bo = np.asarray(bo, np.float32)
    extra = bv @ np.asarray(Wo, np.float32).T + bo  # exact linear fold
    out = np.empty((B, N, D), np.float32)
    for b in range(B):
        out[b] = res.results[2 * b]["o"] + res.results[2 * b + 1]["o"] + extra
    return out, res


def kernel(x, Wq, bq, Wk, bk, Wv, bv, Wo, bo):
    out, _ = _run(x, Wq, bq, Wk, bk, Wv, bv, Wo, bo, trace=False)
    return out
